# revision 9
# baseline (speedup 1.0000x reference)
"""Trainium2 Bass kernel for nn_BinaryLinear: out = sign(x @ sign(W).T + bias).

Strategy
--------
Data-parallel over the 8192-token dim: each of the 8 cores gets 1024 tokens
and the full weight matrix.

On-chip compute (per core) is the NT GEMM z.T = sign(W) @ x.T on the
TensorEngine with the contraction (in_features) on the partition dim:

  psum[outf, tok] = sum_k w_b_T[k, outf] * x_T[k, tok]

Precision/speed: a SINGLE fp16 matmul pass. The PE datapath is FP22
(e10m11), so fp16 x (11 significant bits, host RTN) converts exactly and
streams at 1 PE-cycle/contraction-row — vs 1.5+ for the previous
fp16+fp8-DoubleRow split (DoubleRow only reaches ~1.44x over bf16 on real
HW, so the split's lo pass cost far more than its nominal 0.5 cyc/row).
Outputs are signs, so correctness is a flip count: 11-bit x flips 2198 of
33.5M signs on the seed-0 problem (rel_err = 2*sqrt(flips/N) = 1.62e-2)
vs the 3355 flips the 2e-2 rel-err gate allows. A 12-bit fp32r variant
(rel 1.14e-2) measured 501us vs 465us here: fp32r pays a serialized
~187ns 4-byte weight reload per matmul (227ns cadence vs 216ns).

sign(W) is shipped as fp8e4 (+-1 exact, 1 byte) and used directly as the
stationary operand (mixed fp8/fp16 operands are legal below fp32; FWL keeps
the 128-col weight loads off the critical path). Bias-add + sign +
PSUM->SBUF fuse into one ScalarE activation per [128,512] tile. Output is
written as z.T [out_features, tokens] per core and untransposed on the host.

Startup: the 8MB resident-x stream takes ~23us while one 128-outf weight
block holds only ~14us of PE work, so the first PH=2 weight blocks ride the
gpsimd queue interleaved with the leading x chunks and their k-loops are
fused into one 4-PSUM-bank phase: the PE then has ~0.86us of work per
~0.72us x-chunk arrival and never starves. Measured: first matmul at
~13.5us (NEFF preamble + DMA spin-up), PE idle <1us, 216ns/matmul cadence,
~465us total vs the ~437us pure-PE roofline of this scheme.
"""

import numpy as np
import ml_dtypes

import concourse.tile as tile
import concourse.mybir as mybir
from concourse import bacc
from concourse.bass_utils import run_bass_kernel_spmd

N_CORES = 8
N_TOK = 8192
D_IN = 4096
D_OUT = 4096
P = 128
T = N_TOK // N_CORES  # 1024 tokens per core
KT = D_IN // P  # 32 contraction tiles
MB = D_OUT // P  # 32 weight blocks of 128 out-features
TB = 512  # token block (one PSUM bank of fp32)
NB = T // TB  # 2 token blocks per core
PH = 2  # weight blocks fused into the startup phase

F32 = mybir.dt.float32
FP16 = mybir.dt.float16
FP8 = mybir.dt.float8e4
SIGN = mybir.ActivationFunctionType.Sign
E4M3 = ml_dtypes.float8_e4m3

_nc_cache = None


def build():
    """Build + compile the per-core Bass/Tile module (SPMD: same on all cores)."""
    global _nc_cache
    if _nc_cache is not None:
        return _nc_cache
    nc = bacc.Bacc("TRN2", target_bir_lowering=False, debug=False, num_devices=N_CORES)
    x_d = nc.dram_tensor("x_t", [D_IN, T], FP16, kind="ExternalInput").ap()
    w_d = nc.dram_tensor("w8b", [MB * P, KT * P], FP8, kind="ExternalInput").ap()
    b_d = nc.dram_tensor("bias", [D_OUT], F32, kind="ExternalInput").ap()
    out_d = nc.dram_tensor("out_t", [D_OUT, T], F32, kind="ExternalOutput").ap()

    with tile.TileContext(nc) as tc:
        with (
            tc.tile_pool(name="x", bufs=1) as x_pool,
            tc.tile_pool(name="w8", bufs=3) as w8_pool,
            tc.tile_pool(name="bias", bufs=1) as b_pool,
            tc.tile_pool(name="out", bufs=6) as out_pool,
            tc.tile_pool(name="psum", bufs=8, space="PSUM") as psum_pool,
        ):
            nsls = [slice(n * TB, (n + 1) * TB) for n in range(NB)]

            def stage_w(mb, queue):
                w8s = w8_pool.tile([P, KT * P], FP8, tag="w8", name=f"w8_{mb}")
                queue.dma_start(w8s[:], w_d[mb * P : (mb + 1) * P, :])
                return w8s

            def make_psums(mb):
                return [
                    psum_pool.tile([P, TB], F32, tag="psum", name=f"ps_{mb}_{n}")
                    for n in range(NB)
                ]

            def epilogue(mb, psums):
                for n in range(NB):
                    osb = out_pool.tile([P, TB], F32, tag="osb",
                                        name=f"osb_{mb}_{n}")
                    nc.scalar.activation(
                        osb[:], psums[n][:], SIGN,
                        bias=bias_sb[:, mb : mb + 1],
                    )
                    nc.sync.dma_start(
                        out_d[mb * P : (mb + 1) * P, nsls[n]], osb[:]
                    )

            # Resident x, chunked per k-tile, on the gpsimd DMA queue; the
            # startup-phase W blocks are interleaved ahead of it so the first
            # matmuls are not starved behind the 8MB x stream. The sync queue
            # (bias, remaining W blocks, outputs) is held back by w8-pool
            # backpressure, so it steals little bandwidth early.
            xs = [None] * KT
            w8s_ph = {}

            def stage_x(ko):
                th = x_pool.tile([P, T], FP16, tag=f"x_{ko}", name=f"x_{ko}")
                nc.gpsimd.dma_start(th[:], x_d[ko * P : (ko + 1) * P, :])
                xs[ko] = th

            # (A PE warm-up bridge of dummy matmuls, DMA k-slicing of the
            # first weight blocks, and a pipelined last-block epilogue were
            # all tried and measured neutral-to-worse: first-DMA-completion
            # time varies 13.5-16us run-to-run, which dwarfs those effects,
            # and any PE idle gap after a warm-up lets the HAM clock gate
            # re-throttle. The plain form below measured best.)
            for mb in range(PH):
                w8s_ph[mb] = stage_w(mb, nc.gpsimd)
                stage_x(mb)
            for ko in range(PH, KT):
                stage_x(ko)

            # bias, outf-partition-major: bias_sb[p, mo] = bias[mo*128 + p]
            bias_sb = b_pool.tile([P, MB], F32, tag="bias")
            nc.sync.dma_start(bias_sb[:], b_d.rearrange("(mo p) -> p mo", p=P))

            # Phase 1: blocks 0..PH-1 fused over one k-loop (PH*NB PSUM banks)
            ph_psums = {mb: make_psums(mb) for mb in range(PH)}
            for k in range(KT):
                for mb in range(PH):
                    wk = w8s_ph[mb][:, k * P : (k + 1) * P]
                    for n in range(NB):
                        nc.tensor.matmul(
                            ph_psums[mb][n][:], wk, xs[k][:, nsls[n]],
                            start=(k == 0), stop=(k == KT - 1),
                        )
            for mb in range(PH):
                epilogue(mb, ph_psums[mb])

            # Phase 2: remaining blocks, one at a time (W streamed on sync)
            for mb in range(PH, MB):
                w8s = stage_w(mb, nc.sync)
                psums = make_psums(mb)
                for k in range(KT):
                    wk = w8s[:, k * P : (k + 1) * P]
                    for n in range(NB):
                        nc.tensor.matmul(
                            psums[n][:], wk, xs[k][:, nsls[n]],
                            start=(k == 0), stop=(k == KT - 1),
                        )
                epilogue(mb, psums)
    nc.compile()
    _nc_cache = nc
    return nc


def prep_in_maps(x, weight, bias):
    """Host-side layout prep: fp16 cast of x, transposes, token shards,
    sign(W) -> fp8 in the stationary (lhsT) block layout."""
    x = np.asarray(x, dtype=np.float32)
    weight = np.asarray(weight, dtype=np.float32)
    bias = np.asarray(bias, dtype=np.float32)

    x_t = np.ascontiguousarray(x.T.astype(np.float16))  # [D_IN, N_TOK]

    # lhsT block layout: w8b[mb*128 + p, k*128 + j] = sign(W)[mb*128 + j, k*128 + p]
    wt = np.sign(weight).T.astype(np.float32)  # [D_IN, D_OUT] = [k, outf]
    w8b = np.ascontiguousarray(
        wt.reshape(KT, P, MB, P).transpose(2, 1, 0, 3).reshape(MB * P, KT * P)
    ).astype(E4M3)

    in_maps = []
    for c in range(N_CORES):
        sl = slice(c * T, (c + 1) * T)
        in_maps.append(
            {
                "x_t": np.ascontiguousarray(x_t[:, sl]),
                "w8b": w8b,
                "bias": bias,
            }
        )
    return in_maps


def run(x, weight, bias, **spmd_kwargs):
    """Run on the 8 cores; returns (full_output, BassKernelResults)."""
    nc = build()
    in_maps = prep_in_maps(x, weight, bias)
    res = run_bass_kernel_spmd(nc, in_maps, core_ids=list(range(N_CORES)), **spmd_kwargs)
    out = np.empty((N_TOK, D_OUT), dtype=np.float32)
    for c in range(N_CORES):
        out[c * T : (c + 1) * T, :] = res.results[c]["out_t"].T
    return out, res


def kernel(x, weight, bias):
    out, _ = run(x, weight, bias)
    return out
